# revision 11
# baseline (speedup 1.0000x reference)
"""Trainium2 Bass kernel for nn_AttentionBlock (B=2, T=2048, D=1024, N=16, H=64).

Sharding: tensor-parallel over heads — each of the 8 cores owns 2 heads
(a 128-wide feature slice of q/k/v) and a 128-row slice of w_proj.
Each core computes the qkv projection (feature-major), causal attention for
its heads, and a partial out-projection; the host sums the 8 partials.

All matmuls run as float32r (TF32-like, full PE rate at moving dim >= 256).
Scores are computed transposed (tkv on partitions, tq free) so the softmax
denominator comes from a ones column appended to V in the AV matmul, and exp
needs no max subtraction (|scores| <= ~4 for this data distribution).

Layout trick: the augmented V tile per tkv-chunk has columns
  [ v_h0 (0:64) | ones (64) | zeros (65:128) | v_h1 (128:192) ]
head0 AV uses cols 0:65   (M=65):  psum rows 0:64 = attn_h0, row 64 = sums_h0
head1 AV uses cols 64:192 (M=128): psum row 0 = sums_h1, rows 64:128 = attn_h1
so both heads' attention rows land at the PSUM partitions where the
(partition-locked) vector engine needs them, with no cross-partition moves.
"""

import sys
from contextlib import ExitStack

import numpy as np

for _p in ("/opt/trn_rl_repo", "/root/.axon_site/_ro/trn_rl_repo"):
    if _p not in sys.path:
        sys.path.append(_p)

import concourse.bass as bass
import concourse.mybir as mybir
import concourse.tile as tile
from concourse import bacc
from concourse.bass_utils import run_bass_kernel_spmd

F32 = mybir.dt.float32
F32R = mybir.dt.float32r

B, T, D, N = 2, 2048, 1024, 16
H = 64          # head dim
NC_CORES = 8
BT = B * T      # 4096
TQ = 512        # query-chunk (moving dim)
NQC = T // TQ   # 4 query chunks per batch
DCH = D // 128  # 8 contraction chunks for projections
SUB = 4         # tkv sub-batch (d-chunks) between MM1/MM2 groups

_CACHED = {}


def _build_program():
    nc = bacc.Bacc("TRN2", target_bir_lowering=False, debug=False,
                   num_devices=NC_CORES)

    xT_d = nc.declare_dram_parameter("xT", [D, BT], F32R, isOutput=False)
    wqkv_d = nc.declare_dram_parameter("wqkv", [D, 384], F32R, isOutput=False)
    bqkv_d = nc.declare_dram_parameter("bqkv", [384], F32, isOutput=False)
    wproj_d = nc.declare_dram_parameter("wproj", [128, D], F32R, isOutput=False)
    masks_d = nc.declare_dram_parameter("masks", [128, 4 * TQ], F32, isOutput=False)
    ident_d = nc.declare_dram_parameter("ident", [128, 128], F32, isOutput=False)
    vfill_d = nc.declare_dram_parameter("vfill", [128, 1024], F32R, isOutput=False)
    sel_d = nc.declare_dram_parameter("sel", [65, 128], F32R, isOutput=False)
    out_d = nc.declare_dram_parameter("out_part", [BT, D], F32, isOutput=True)
    kT_d = nc.declare_dram_parameter("kT_out", [128, BT], F32R, isOutput=True)
    vT_d = nc.declare_dram_parameter("vT_out", [128, BT], F32, isOutput=True)

    EXP = mybir.ActivationFunctionType.Exp

    with tile.TileContext(nc) as tc, ExitStack() as ctx, \
            nc.allow_low_precision(reason="float32r tiles store full fp32"):
        const = ctx.enter_context(tc.tile_pool(name="const", bufs=1))
        xpool = ctx.enter_context(tc.tile_pool(name="x", bufs=2))
        qkv_pool = ctx.enter_context(tc.tile_pool(name="qkv", bufs=2))
        vaug_pool = ctx.enter_context(tc.tile_pool(name="vaug", bufs=2))
        epool = ctx.enter_context(tc.tile_pool(name="e", bufs=14))
        apool = ctx.enter_context(tc.tile_pool(name="attnT", bufs=2))
        spool = ctx.enter_context(tc.tile_pool(name="small", bufs=2))
        opool = ctx.enter_context(tc.tile_pool(name="of", bufs=3))
        PSUM = bass.MemorySpace.PSUM
        psQ = ctx.enter_context(tc.tile_pool(name="psQ", bufs=2, space=PSUM))
        psT = ctx.enter_context(tc.tile_pool(name="psT", bufs=1, space=PSUM))
        psS = ctx.enter_context(tc.tile_pool(name="psS", bufs=2, space=PSUM))
        psO = ctx.enter_context(tc.tile_pool(name="psO", bufs=2, space=PSUM))

        # ---- constants ----
        wq_sb = const.tile([128, DCH, 384], F32R)
        nc.sync.dma_start(wq_sb[:], wqkv_d[:].rearrange("(d p) f -> p d f", p=128))
        bq_sb = const.tile([128, 3], F32)
        nc.sync.dma_start(bq_sb[:], bqkv_d[:].rearrange("(g p) -> p g", p=128))
        wp_sb = const.tile([128, D], F32R)
        nc.sync.dma_start(wp_sb[:], wproj_d[:])
        mask_sb = const.tile([128, 4, TQ], F32)
        nc.sync.dma_start(mask_sb[:], masks_d[:].rearrange("p (s j) -> p s j", s=4))
        id_sb = const.tile([128, 128], F32)
        nc.sync.dma_start(id_sb[:], ident_d[:])
        zbias = const.tile([128, 1], F32)
        nc.vector.memset(zbias[:], 0.0)
        sel_sb = const.tile([65, 128], F32R)
        nc.sync.dma_start(sel_sb[:], sel_d[:])

        for b in range(B):
            # ================= stage P: qkv projection (feature-major) ======
            qT = qkv_pool.tile([128, T], F32R, tag="qT")
            kT = qkv_pool.tile([128, T], F32R, tag="kT")
            vT = qkv_pool.tile([128, T], F32, tag="vT")
            vaug = vaug_pool.tile([128, T // 128, 192], F32R, tag="vaug")
            attnT = apool.tile([128, T], F32R, tag="attnT")
            # cols 64:128 = [ones | zeros*63] via DMA (f32r memset fails ISA)
            nc.sync.dma_start(
                vaug[:, :, 64:128],
                vfill_d[:].rearrange("p (c f) -> p c f", c=T // 128),
            )
            tgt = {0: qT, 1: kT, 2: vT}

            for n in range(NQC):
                xt = xpool.tile([128, DCH, TQ], F32R, tag="xt")
                col0 = b * T + n * TQ
                nc.sync.dma_start(
                    xt[:],
                    xT_d[:].rearrange("(d p) j -> p d j", p=128)[:, :, col0:col0 + TQ],
                )
                for g in range(3):
                    ps = psQ.tile([128, TQ], F32, tag="ps")
                    for d in range(DCH):
                        nc.tensor.matmul(
                            ps[:],
                            lhsT=wq_sb[:, d, g * 128:(g + 1) * 128],
                            rhs=xt[:, d, :],
                            start=(d == 0),
                            stop=(d == DCH - 1),
                        )
                    nc.vector.tensor_scalar_add(
                        tgt[g][:, n * TQ:(n + 1) * TQ], ps[:], bq_sb[:, g:g + 1]
                    )
                # v -> token-major (both heads in one 128x128 transpose)
                for m in range(TQ // 128):
                    tp = psT.tile([128, 512], F32, tag="tp")
                    cidx = n * (TQ // 128) + m
                    nc.tensor.transpose(
                        tp[:, 0:128], vT[:, cidx * 128:(cidx + 1) * 128], id_sb[:]
                    )
                    nc.any.tensor_copy(vaug[:, cidx, 0:64], tp[:, 0:64])
                    nc.any.tensor_copy(vaug[:, cidx, 128:192], tp[:, 64:128])

            nc.sync.dma_start(kT_d[:, b * T:(b + 1) * T], kT[:])
            nc.sync.dma_start(vT_d[:, b * T:(b + 1) * T], vT[:])

            # ================= stage A: attention =================
            for n in range(NQC):
                Dn = (n + 1) * (TQ // 128)  # causal: tkv chunks needed
                oh = [psO.tile([128, TQ], F32, tag="o", name=f"oh{h}_{b}_{n}")
                      for h in range(2)]
                for d0 in range(0, Dn, SUB):
                    dhi = min(d0 + SUB, Dn)
                    ets = {}
                    for d in range(d0, dhi):
                        for h in range(2):
                            sp = psS.tile([128, TQ], F32, tag="s")
                            nc.tensor.matmul(
                                sp[:],
                                lhsT=kT[h * 64:(h + 1) * 64,
                                        d * 128:(d + 1) * 128],
                                rhs=qT[h * 64:(h + 1) * 64,
                                       n * TQ:(n + 1) * TQ],
                                start=True, stop=True,
                            )
                            et = epool.tile([128, TQ], F32R, tag="e")
                            nc.scalar.activation(et[:], sp[:], EXP, bias=zbias[:])
                            s_idx = d - n * (TQ // 128)
                            if s_idx >= 0:  # diagonal block: apply causal mask
                                nc.vector.tensor_mul(
                                    et[:], et[:], mask_sb[:, s_idx, :]
                                )
                            ets[(d, h)] = et
                    for d in range(d0, dhi):
                        # head0: cols [v_h0 | ones] -> rows 0:64 attn, 64 sums
                        nc.tensor.matmul(
                            oh[0][0:65, :],
                            lhsT=vaug[:, d, 0:65],
                            rhs=ets[(d, 0)][:],
                            start=(d == 0), stop=(d == Dn - 1),
                        )
                        # head1: cols [ones | zeros*63 | v_h1] -> row 0 sums,
                        # rows 64:128 attn
                        nc.tensor.matmul(
                            oh[1][0:128, :],
                            lhsT=vaug[:, d, 64:192],
                            rhs=ets[(d, 1)][:],
                            start=(d == 0), stop=(d == Dn - 1),
                        )
                # softmax normalization: attn rows / sums row, per head.
                # ss rows 0:64 <- oh1 rows 0:64 (row 0 = sums_h1, rest finite
                # junk), row 64 <- sums_h0; sel zeros the junk rows and
                # broadcasts sums_h0 to psum rows 0:64, sums_h1 to 64:128.
                ss = spool.tile([65, TQ], F32R, tag="ss")
                nc.any.tensor_copy(ss[0:64, :], oh[1][0:64, :])
                nc.any.tensor_copy(ss[64:65, :], oh[0][64:65, :])
                bc = psS.tile([128, TQ], F32, tag="s", name=f"bc_{b}_{n}")
                nc.tensor.matmul(bc[:], lhsT=sel_sb[:], rhs=ss[:],
                                 start=True, stop=True)
                bcs = spool.tile([128, TQ], F32, tag="bcs")
                nc.vector.reciprocal(bcs[:], bc[:])
                nc.vector.tensor_mul(
                    attnT[0:64, n * TQ:(n + 1) * TQ], oh[0][0:64, :], bcs[0:64, :]
                )
                nc.vector.tensor_mul(
                    attnT[64:128, n * TQ:(n + 1) * TQ], oh[1][64:128, :],
                    bcs[64:128, :]
                )
                # ============ stage O: partial out-projection ============
                for m in range(TQ // 128):
                    for e in range(D // 512):
                        pf = psS.tile([128, 512], F32, tag="s")
                        nc.tensor.matmul(
                            pf[:],
                            lhsT=attnT[:, n * TQ + m * 128:
                                       n * TQ + (m + 1) * 128],
                            rhs=wp_sb[:, e * 512:(e + 1) * 512],
                            start=True, stop=True,
                        )
                        of = opool.tile([128, 512], F32, tag="of")
                        nc.any.tensor_copy(of[:], pf[:])
                        nc.sync.dma_start(
                            out_d[b * T + n * TQ + m * 128:
                                  b * T + n * TQ + (m + 1) * 128,
                                  e * 512:(e + 1) * 512],
                            of[:],
                        )

    nc.compile()
    return nc


def _host_prep(x, w_attn, b_attn, w_proj):
    xT = np.ascontiguousarray(x.reshape(BT, D).T).astype(np.float32)
    scale = 1.0 / np.sqrt(H)

    # causal masks for the 4 diagonal-block offsets: keep iff i <= j - 128*s
    i = np.arange(128)[:, None]
    j = np.arange(TQ)[None, :]
    masks = np.concatenate(
        [(i <= j - 128 * s).astype(np.float32) for s in range(4)], axis=1
    )  # (128, 2048)
    ident = np.eye(128, dtype=np.float32)
    vfill = np.zeros((128, 16, 64), np.float32)
    vfill[:, :, 0] = 1.0
    vfill = vfill.reshape(128, 1024)
    sel = np.zeros((65, 128), np.float32)
    sel[64, 0:64] = 1.0   # sums_h0 -> bcast rows 0:64
    sel[0, 64:128] = 1.0  # sums_h1 -> bcast rows 64:128


    in_maps = []
    for c in range(NC_CORES):
        fs = slice(c * 128, (c + 1) * 128)
        wq = w_attn[:, 0 * D:1 * D][:, fs] * scale
        wk = w_attn[:, 1 * D:2 * D][:, fs]
        wv = w_attn[:, 2 * D:3 * D][:, fs]
        wqkv = np.concatenate([wq, wk, wv], axis=1).astype(np.float32)  # (1024,384)
        bqkv = np.concatenate([
            b_attn[0 * D:1 * D][fs] * scale,
            b_attn[1 * D:2 * D][fs],
            b_attn[2 * D:3 * D][fs],
        ]).astype(np.float32)
        wp = np.ascontiguousarray(w_proj[fs, :]).astype(np.float32)  # (128,1024)
        in_maps.append({
            "xT": xT, "wqkv": wqkv, "bqkv": bqkv, "wproj": wp,
            "masks": masks, "ident": ident, "vfill": vfill, "sel": sel,
        })
    return in_maps


def gather(results, b_proj):
    out = np.zeros((BT, D), np.float64)
    for r in results:
        out += r["out_part"]
    out = (out + b_proj).astype(np.float32).reshape(B, T, D)

    k = np.zeros((B, T, N, H), np.float32)
    v = np.zeros((B, T, N, H), np.float32)
    for c, r in enumerate(results):
        # (128, B*T) -> (2 heads, 64, B, T) -> (B, T, 2, 64)
        kr = r["kT_out"].reshape(2, H, B, T).transpose(2, 3, 0, 1)
        vr = r["vT_out"].reshape(2, H, B, T).transpose(2, 3, 0, 1)
        k[:, :, 2 * c:2 * c + 2, :] = kr
        v[:, :, 2 * c:2 * c + 2, :] = vr
    return out, k, v


def run(inputs, trace=False):
    """Runs the SPMD kernel; returns ((out, k, v), BassKernelResults)."""
    if "nc" not in _CACHED:
        _CACHED["nc"] = _build_program()
    nc = _CACHED["nc"]

    x = np.asarray(inputs["x"], np.float32)
    w_attn = np.asarray(inputs["w_attn"], np.float32)
    b_attn = np.asarray(inputs["b_attn"], np.float32)
    w_proj = np.asarray(inputs["w_proj"], np.float32)
    b_proj = np.asarray(inputs["b_proj"], np.float32)

    in_maps = _host_prep(x, w_attn, b_attn, w_proj)
    res = run_bass_kernel_spmd(nc, in_maps, list(range(NC_CORES)), trace=trace)
    return gather(res.results, b_proj), res


def kernel(**inputs):
    (out, k, v), _ = run(inputs)
    return out, k, v


# revision 23
# speedup vs baseline: 1.6481x; 1.6481x over previous
"""Trainium2 Bass kernel for nn_AttentionBlock (B=2, T=2048, D=1024, N=16, H=64).

Sharding: tensor-parallel over heads — each of the 8 cores owns 2 heads
(a 128-wide feature slice of q/k/v) and a 128-row slice of w_proj.
Each core computes the qkv projection (feature-major), causal attention for
its heads, and a partial out-projection; the host sums the 8 partials.

All matmuls run as float32r (TF32-like, full PE rate at moving dim >= 256).
Scores are computed transposed (tkv on partitions, tq free) so the softmax
denominator comes from a ones column appended to V in the AV matmul, and exp
needs no max subtraction (|scores| <= ~4 for this data distribution).

Layout trick: the augmented V tile per tkv-chunk has columns
  [ v_h0 (0:64) | ones (64) | zeros (65:128) | v_h1 (128:192) ]
head0 AV uses cols 0:65   (M=65):  psum rows 0:64 = attn_h0, row 64 = sums_h0
head1 AV uses cols 64:192 (M=128): psum row 0 = sums_h1, rows 64:128 = attn_h1
so both heads' attention rows land at the PSUM partitions where the
(partition-locked) vector engine needs them, with no cross-partition moves.
"""

import sys
from contextlib import ExitStack

import numpy as np

for _p in ("/opt/trn_rl_repo", "/root/.axon_site/_ro/trn_rl_repo"):
    if _p not in sys.path:
        sys.path.append(_p)

import concourse.bass as bass
import concourse.mybir as mybir
import concourse.tile as tile
from concourse import bacc
from concourse.bass_utils import run_bass_kernel_spmd

F32 = mybir.dt.float32
F32R = mybir.dt.float32r
BF16 = mybir.dt.bfloat16

B, T, D, N = 2, 2048, 1024, 16
H = 64          # head dim
NC_CORES = 8
BT = B * T      # 4096
TQ = 512        # query-chunk (moving dim)
NQC = T // TQ   # 4 query chunks per batch
DCH = D // 128  # 8 contraction chunks for projections
SUB = 8         # tkv sub-batch (d-chunks) between MM1/MM2 groups

_CACHED = {}


def _build_program():
    nc = bacc.Bacc("TRN2", target_bir_lowering=False, debug=False,
                   num_devices=NC_CORES)

    xT_d = nc.declare_dram_parameter("xT", [D, BT], BF16, isOutput=False)
    wqkv_d = nc.declare_dram_parameter("wqkv", [D, 384], BF16, isOutput=False)
    bqkv_d = nc.declare_dram_parameter("bqkv", [384], F32, isOutput=False)
    wproj_d = nc.declare_dram_parameter("wproj", [128, D], F32R, isOutput=False)
    masks_d = nc.declare_dram_parameter("masks", [128, 4 * TQ], BF16, isOutput=False)
    ident_d = nc.declare_dram_parameter("ident", [128, 128], F32, isOutput=False)
    vfill_d = nc.declare_dram_parameter("vfill", [128, 1024], BF16, isOutput=False)
    sel_d = nc.declare_dram_parameter("sel", [65, 128], F32R, isOutput=False)
    zpad_d = nc.declare_dram_parameter("zpad", [64, T], F32R, isOutput=False)
    out_d = nc.declare_dram_parameter("out_part", [BT, D], F32, isOutput=True)
    kT_d = nc.declare_dram_parameter("kT_out", [128, BT], F32R, isOutput=True)
    vT_d = nc.declare_dram_parameter("vT_out", [128, BT], F32, isOutput=True)

    EXP = mybir.ActivationFunctionType.Exp

    with tile.TileContext(nc) as tc, ExitStack() as ctx, \
            nc.allow_low_precision(reason="float32r tiles store full fp32"):
        const = ctx.enter_context(tc.tile_pool(name="const", bufs=1))
        xpool = ctx.enter_context(tc.tile_pool(name="x", bufs=3))
        qkv_pool = ctx.enter_context(tc.tile_pool(name="qkv", bufs=2))
        vaug_pool = ctx.enter_context(tc.tile_pool(name="vaug", bufs=2))
        epool = ctx.enter_context(tc.tile_pool(name="e", bufs=24))
        apool = ctx.enter_context(tc.tile_pool(name="attnT", bufs=2))
        spool = ctx.enter_context(tc.tile_pool(name="small", bufs=2))
        opool = ctx.enter_context(tc.tile_pool(name="of", bufs=6))
        PSUM = bass.MemorySpace.PSUM
        psQ = ctx.enter_context(tc.tile_pool(name="psQ", bufs=2, space=PSUM))
        psS = ctx.enter_context(tc.tile_pool(name="psS", bufs=2, space=PSUM))
        psO = ctx.enter_context(tc.tile_pool(name="psO", bufs=4, space=PSUM))

        # ---- constants ----
        wq_sb = const.tile([128, DCH, 384], BF16)
        nc.sync.dma_start(wq_sb[:], wqkv_d[:].rearrange("(d p) f -> p d f", p=128))
        bq_sb = const.tile([128, 3], F32)
        nc.sync.dma_start(bq_sb[:], bqkv_d[:].rearrange("(g p) -> p g", p=128))
        wp_sb = const.tile([128, D], F32R)
        nc.sync.dma_start(wp_sb[:], wproj_d[:])
        mask_sb = const.tile([128, 4, TQ], BF16)
        nc.sync.dma_start(mask_sb[:], masks_d[:].rearrange("p (s j) -> p s j", s=4))
        id_sb = const.tile([128, 128], F32)
        nc.sync.dma_start(id_sb[:], ident_d[:])
        zbias = const.tile([128, 1], F32)
        nc.vector.memset(zbias[:], 0.0)
        sel_sb = const.tile([65, 128], F32R)
        nc.sync.dma_start(sel_sb[:], sel_d[:])

        for b in range(B):
            # ================= stage P: qkv projection (feature-major) ======
            qT = qkv_pool.tile([128, T], F32R, tag="qT")
            kTp0 = qkv_pool.tile([128, T], F32R, tag="kTp0")
            kTp1 = qkv_pool.tile([128, T], F32R, tag="kTp1")
            vT = qkv_pool.tile([128, T], F32, tag="vT")
            nc.sync.dma_start(kTp0[64:128, :], zpad_d[:])
            nc.sync.dma_start(kTp1[0:64, :], zpad_d[:])
            vaug = vaug_pool.tile([128, T // 128, 192], BF16, tag="vaug")
            attnT = apool.tile([128, T], F32R, tag="attnT")
            # cols 64:128 = [ones | zeros*63] via DMA (f32r memset fails ISA)
            nc.sync.dma_start(
                vaug[:, :, 64:128],
                vfill_d[:].rearrange("p (c f) -> p c f", c=T // 128),
            )
            tgt = {0: qT, 2: vT}

            for n in range(NQC):
                xt = xpool.tile([128, DCH, TQ], BF16, tag="xt")
                col0 = b * T + n * TQ
                nc.sync.dma_start(
                    xt[:],
                    xT_d[:].rearrange("(d p) j -> p d j", p=128)[:, :, col0:col0 + TQ],
                )
                for g in range(3):
                    ps = psQ.tile([128, TQ], F32, tag="ps")
                    for d in range(DCH):
                        nc.tensor.matmul(
                            ps[:],
                            lhsT=wq_sb[:, d, g * 128:(g + 1) * 128],
                            rhs=xt[:, d, :],
                            start=(d == 0),
                            stop=(d == DCH - 1),
                        )
                    if g == 1:
                        nc.vector.tensor_scalar_add(
                            kTp0[0:64, n * TQ:(n + 1) * TQ], ps[0:64, :],
                            bq_sb[0:64, g:g + 1]
                        )
                        nc.vector.tensor_scalar_add(
                            kTp1[64:128, n * TQ:(n + 1) * TQ], ps[64:128, :],
                            bq_sb[64:128, g:g + 1]
                        )
                    else:
                        nc.vector.tensor_scalar_add(
                            tgt[g][:, n * TQ:(n + 1) * TQ], ps[:], bq_sb[:, g:g + 1]
                        )
                # v -> token-major (both heads in one 128x128 transpose)
                for m in range(TQ // 128):
                    tp = psQ.tile([128, 512], F32, tag="ps", name=f"tp_{b}_{n}_{m}")
                    cidx = n * (TQ // 128) + m
                    nc.tensor.transpose(
                        tp[:, 0:128], vT[:, cidx * 128:(cidx + 1) * 128], id_sb[:]
                    )
                    nc.vector.tensor_copy(vaug[:, cidx, 0:64], tp[:, 0:64])
                    nc.vector.tensor_copy(vaug[:, cidx, 128:192], tp[:, 64:128])

            nc.sync.dma_start(kT_d[0:64, b * T:(b + 1) * T], kTp0[0:64, :])
            nc.sync.dma_start(kT_d[64:128, b * T:(b + 1) * T], kTp1[64:128, :])
            nc.sync.dma_start(vT_d[:, b * T:(b + 1) * T], vT[:])

            # ================= stage A: attention =================
            # The normalization + out-projection of chunk n-1 are emitted
            # inside chunk n's d-loop (software pipeline) so the PE stream
            # never waits on the DVE normalization chain.
            def norm_part1(pd):
                b_, n_, oh_, ss_ = pd
                bc = psS.tile([128, TQ], F32, tag="s", name=f"bc_{b_}_{n_}")
                nc.tensor.matmul(bc[:], lhsT=sel_sb[:], rhs=ss_[:],
                                 start=True, stop=True)
                # copy PSUM->SBUF first so the bank frees before the slow
                # reciprocal
                bcc = spool.tile([128, TQ], F32, tag="bcc",
                                 name=f"bcc_{b_}_{n_}")
                nc.vector.tensor_copy(bcc[:], bc[:])
                bcs = spool.tile([128, TQ], F32, tag="bcs",
                                 name=f"bcs_{b_}_{n_}")
                nc.vector.reciprocal(bcs[:], bcc[:])
                pd.append(bcs)

            def norm_part2(pd):
                b_, n_, oh_, ss_, bcs = pd
                nc.vector.tensor_mul(
                    attnT[0:64, n_ * TQ:(n_ + 1) * TQ], oh_[0][0:64, :],
                    bcs[0:64, :]
                )
                nc.vector.tensor_mul(
                    attnT[64:128, n_ * TQ:(n_ + 1) * TQ], oh_[1][64:128, :],
                    bcs[64:128, :]
                )
                for m in range(TQ // 128):
                    for e in range(D // 512):
                        pf = psQ.tile([128, 512], F32, tag="ps",
                                      name=f"pf_{b_}_{n_}_{m}_{e}")
                        nc.tensor.matmul(
                            pf[:],
                            lhsT=attnT[:, n_ * TQ + m * 128:
                                       n_ * TQ + (m + 1) * 128],
                            rhs=wp_sb[:, e * 512:(e + 1) * 512],
                            start=True, stop=True,
                        )
                        of = opool.tile([128, 512], F32, tag="of",
                                        name=f"of_{b_}_{n_}_{m}_{e}")
                        if e == 0:
                            nc.vector.tensor_copy(of[:], pf[:])
                        else:
                            nc.scalar.copy(of[:], pf[:])
                        nc.sync.dma_start(
                            out_d[b_ * T + n_ * TQ + m * 128:
                                  b_ * T + n_ * TQ + (m + 1) * 128,
                                  e * 512:(e + 1) * 512],
                            of[:],
                        )

            pend = None
            for n in range(NQC):
                Dn = (n + 1) * (TQ // 128)  # causal: tkv chunks needed
                oh = [psO.tile([128, TQ], F32, tag="o", name=f"oh{h}_{b}_{n}")
                      for h in range(2)]
                for idx, d0 in enumerate(range(0, Dn, SUB)):
                    dhi = min(d0 + SUB, Dn)
                    ets = {}
                    for d in range(d0, dhi):
                        for h in range(2):
                            kTp = kTp0 if h == 0 else kTp1
                            sp = psS.tile([128, TQ], F32, tag="s")
                            nc.tensor.matmul(
                                sp[:],
                                lhsT=kTp[:, d * 128:(d + 1) * 128],
                                rhs=qT[:, n * TQ:(n + 1) * TQ],
                                start=True, stop=True,
                            )
                            et = epool.tile([128, TQ], BF16, tag="e")
                            s_idx = d - n * (TQ // 128)
                            j0 = max(0, s_idx) * 128  # first live column
                            nc.scalar.activation(et[:, j0:], sp[:, j0:], EXP,
                                                 bias=zbias[:])
                            if s_idx >= 0:  # diagonal block: apply causal mask
                                nc.vector.tensor_mul(
                                    et[:, j0:], et[:, j0:],
                                    mask_sb[:, s_idx, j0:]
                                )
                            ets[(d, h)] = (et, j0)
                    if idx == 0 and pend is not None:
                        norm_part1(pend)
                    for d in range(d0, dhi):
                        et0, j0 = ets[(d, 0)]
                        et1, _ = ets[(d, 1)]
                        # head0: cols [v_h0 | ones] -> rows 0:64 attn, 64 sums
                        nc.tensor.matmul(
                            oh[0][0:65, j0:],
                            lhsT=vaug[:, d, 0:65],
                            rhs=et0[:, j0:],
                            start=(d == 0), stop=(d == Dn - 1),
                        )
                        # head1: cols [ones | zeros*63 | v_h1] -> row 0 sums,
                        # rows 64:128 attn
                        nc.tensor.matmul(
                            oh[1][0:128, j0:],
                            lhsT=vaug[:, d, 64:192],
                            rhs=et1[:, j0:],
                            start=(d == 0), stop=(d == Dn - 1),
                        )
                    if idx == 1 and pend is not None:
                        norm_part2(pend)
                        pend = None
                if pend is not None:  # single-sub-batch chunk (n == 0)
                    norm_part2(pend)
                    pend = None
                # sums rows for this chunk: ss rows 0:64 <- oh1 rows 0:64
                # (row 0 = sums_h1, rest finite junk), row 64 <- sums_h0;
                # sel zeros the junk and broadcasts per head.
                ss = spool.tile([65, TQ], F32R, tag="ss", name=f"ss_{b}_{n}")
                nc.vector.tensor_copy(ss[0:64, :], oh[1][0:64, :])
                nc.vector.tensor_copy(ss[64:65, :], oh[0][64:65, :])
                pend = [b, n, oh, ss]
            norm_part1(pend)
            norm_part2(pend)
            pend = None

    nc.compile()
    return nc


def _host_prep(x, w_attn, b_attn, w_proj):
    import ml_dtypes as _md2
    xT = np.ascontiguousarray(x.reshape(BT, D).T).astype(_md2.bfloat16)
    scale = 1.0 / np.sqrt(H)

    # causal masks for the 4 diagonal-block offsets: keep iff i <= j - 128*s
    i = np.arange(128)[:, None]
    j = np.arange(TQ)[None, :]
    import ml_dtypes
    masks = np.concatenate(
        [(i <= j - 128 * s).astype(np.float32) for s in range(4)], axis=1
    ).astype(ml_dtypes.bfloat16)  # (128, 2048)
    ident = np.eye(128, dtype=np.float32)
    import ml_dtypes as _md
    vfill = np.zeros((128, 16, 64), _md.bfloat16)
    vfill[:, :, 0] = 1.0
    vfill = vfill.reshape(128, 1024)
    zpad = np.zeros((64, 2048), np.float32)
    sel = np.zeros((65, 128), np.float32)
    sel[64, 0:64] = 1.0   # sums_h0 -> bcast rows 0:64
    sel[0, 64:128] = 1.0  # sums_h1 -> bcast rows 64:128


    in_maps = []
    for c in range(NC_CORES):
        fs = slice(c * 128, (c + 1) * 128)
        wq = w_attn[:, 0 * D:1 * D][:, fs] * scale
        wk = w_attn[:, 1 * D:2 * D][:, fs]
        wv = w_attn[:, 2 * D:3 * D][:, fs]
        wqkv = np.concatenate([wq, wk, wv], axis=1).astype(_md2.bfloat16)
        bqkv = np.concatenate([
            b_attn[0 * D:1 * D][fs] * scale,
            b_attn[1 * D:2 * D][fs],
            b_attn[2 * D:3 * D][fs],
        ]).astype(np.float32)
        wp = np.ascontiguousarray(w_proj[fs, :]).astype(np.float32)  # (128,1024)
        in_maps.append({
            "xT": xT, "wqkv": wqkv, "bqkv": bqkv, "wproj": wp,
            "masks": masks, "ident": ident, "vfill": vfill, "sel": sel,
            "zpad": zpad,
        })
    return in_maps


def gather(results, b_proj):
    out = np.zeros((BT, D), np.float64)
    for r in results:
        out += r["out_part"]
    out = (out + b_proj).astype(np.float32).reshape(B, T, D)

    k = np.zeros((B, T, N, H), np.float32)
    v = np.zeros((B, T, N, H), np.float32)
    for c, r in enumerate(results):
        # (128, B*T) -> (2 heads, 64, B, T) -> (B, T, 2, 64)
        kr = r["kT_out"].reshape(2, H, B, T).transpose(2, 3, 0, 1)
        vr = r["vT_out"].reshape(2, H, B, T).transpose(2, 3, 0, 1)
        k[:, :, 2 * c:2 * c + 2, :] = kr
        v[:, :, 2 * c:2 * c + 2, :] = vr
    return out, k, v


def run(inputs, trace=False):
    """Runs the SPMD kernel; returns ((out, k, v), BassKernelResults)."""
    if "nc" not in _CACHED:
        _CACHED["nc"] = _build_program()
    nc = _CACHED["nc"]

    x = np.asarray(inputs["x"], np.float32)
    w_attn = np.asarray(inputs["w_attn"], np.float32)
    b_attn = np.asarray(inputs["b_attn"], np.float32)
    w_proj = np.asarray(inputs["w_proj"], np.float32)
    b_proj = np.asarray(inputs["b_proj"], np.float32)

    in_maps = _host_prep(x, w_attn, b_attn, w_proj)
    res = run_bass_kernel_spmd(nc, in_maps, list(range(NC_CORES)), trace=trace)
    return gather(res.results, b_proj), res


def kernel(**inputs):
    (out, k, v), _ = run(inputs)
    return out, k, v
